# revision 18
# baseline (speedup 1.0000x reference)
"""AttentionHead kernel for 8 trn2 NeuronCores.

Shards the 32 independent (batch n, head h) attention problems across 8
cores (4 pairs per core).  Host-side prep only re-lays-out data: x is
transposed per pair to [E, S] (the PE contracts over the partition dim,
so x must sit with E on partitions) and the 1/sqrt(512) scale is folded
into Wq/bq.

Per core, per (n,h) pair with xT [512, 2048]:
  1. Packed Q/K projection: lhsT=[Wq'|Wk] [128,128] per E-chunk, 4-chunk
     PSUM accumulation -> QT/KT [64, 2048] each (D on partitions).
  2. QT/KT mirrored to partitions 64-127 via SBUF->SBUF DMA so energy
     matmuls can row-pack two K=64 matmuls into the 128-row PE array.
  3. Energy E^T[k, q] = KT-slice.T @ QT-slice, exp on ScalarE straight
     out of PSUM (softmax max-subtraction skipped: |energy| < ~1).
  4. V projection -> VT [65, 2048] (row 64 = ones), PE-transposed to V
     [S, D+1] layout; the ones column makes the attn@V matmul also
     accumulate the softmax denominator as output row 64.
  5. OT [65, 512] PE-transposed back to [q, d]; DVE reciprocal + per-
     partition scale normalizes; DMA out.

Matmul inputs are bf16 (same 1 cycle/row PE rate as fp32r, but with
separate LDWEIGHTS instructions so bacc can distribute semaphore waits —
fused fp32/fp32r matmuls only support a single sync wait and reject
Tile's multi-dep wait sets).  Accumulation is fp32 in PSUM and the
softmax normalization path stays fp32; expected output error vs the
fp32 reference is ~1e-3 relative.
"""

import numpy as np

import concourse.bass as bass
import concourse.mybir as mybir
from concourse.tile import TileContext
from concourse.bass_utils import run_bass_kernel_spmd
from concourse.masks import make_identity

N, S, H, E, D = 4, 2048, 8, 512, 64
NCORES = 8
PAIRS = (N * H) // NCORES  # 4 (n,h) pairs per core
EC = E // 128              # 4 E-chunks
SQT = 512                  # q-slice width (one PSUM bank)
NSQ = S // SQT             # 4 q-slices
NSK = S // 128             # 16 k-tiles
F32 = mybir.dt.float32
BF16 = mybir.dt.bfloat16


def build_bass() -> bass.Bass:
    nc = bass.Bass()

    xt = nc.declare_dram_parameter("xt", [PAIRS, E, S], BF16, isOutput=False)
    wqk = nc.declare_dram_parameter("wqk", [E, 128], BF16, isOutput=False)
    bqk = nc.declare_dram_parameter("bqk", [1, 128], BF16, isOutput=False)
    wkq = nc.declare_dram_parameter("wkq", [E, 128], BF16, isOutput=False)
    bkq = nc.declare_dram_parameter("bkq", [1, 128], BF16, isOutput=False)
    wv = nc.declare_dram_parameter("wv", [E, D], BF16, isOutput=False)
    bv = nc.declare_dram_parameter("bv", [1, D], BF16, isOutput=False)
    out = nc.declare_dram_parameter("out", [PAIRS, S, D], F32, isOutput=True)

    with TileContext(nc) as tc:
        with (
            tc.tile_pool(name="const", bufs=1) as cpool,
            tc.tile_pool(name="xt", bufs=2) as xpool,
            tc.tile_pool(name="qk", bufs=2) as qkpool,
            tc.tile_pool(name="vt", bufs=2) as vtpool,
            tc.tile_pool(name="vaug", bufs=2) as vpool,
            tc.tile_pool(name="expe", bufs=3) as epool,
            tc.tile_pool(name="osb", bufs=2) as opool,
            tc.tile_pool(name="fin", bufs=2) as fpool,
            tc.tile_pool(name="stat", bufs=8) as spool,
            tc.tile_pool(name="pe", bufs=2, space="PSUM") as pe_ps,
            tc.tile_pool(name="ot", bufs=2, space="PSUM") as ot_ps,
            tc.tile_pool(name="misc", bufs=2, space="PSUM") as misc_ps,
        ):
            # ---- constants ----
            wqk_sb = cpool.tile([128, EC, 128], BF16, tag="wqk")
            nc.sync.dma_start(
                out=wqk_sb[:, :, :],
                in_=wqk.rearrange("(c k) m -> k c m", k=128),
            )
            wv_sb = cpool.tile([128, EC, D], BF16, tag="wv")
            nc.sync.dma_start(
                out=wv_sb[:, :, :],
                in_=wv.rearrange("(c k) d -> k c d", k=128),
            )
            wkq_sb = cpool.tile([128, EC, 128], BF16, tag="wkq")
            nc.sync.dma_start(
                out=wkq_sb[:, :, :],
                in_=wkq.rearrange("(c k) m -> k c m", k=128),
            )
            bqk_sb = cpool.tile([1, 128], BF16, tag="bqk")
            nc.sync.dma_start(out=bqk_sb[:, :], in_=bqk[:, :])
            bkq_sb = cpool.tile([1, 128], BF16, tag="bkq")
            nc.sync.dma_start(out=bkq_sb[:, :], in_=bkq[:, :])
            bv_sb = cpool.tile([1, D], BF16, tag="bv")
            nc.sync.dma_start(out=bv_sb[:, :], in_=bv[:, :])
            ones_row = cpool.tile([1, SQT], BF16, tag="ones")
            nc.vector.memset(ones_row[:, :], 1.0)
            ident = cpool.tile([128, 128], F32, tag="ident")
            make_identity(nc, ident[:, :])
            identb = cpool.tile([128, 128], BF16, tag="identb")
            nc.vector.tensor_copy(out=identb[:, :], in_=ident[:, :])
            # DVE join: the fp32 OT-transposes are fused (single-sync-wait)
            # matmults, so identity's last writer must share a semaphore with
            # their other input (DVE).
            nc.vector.tensor_copy(out=ident[:, :], in_=ident[:, :])

            for p in range(PAIRS):
                # ---- load xT for this pair: [128, EC, S] ----
                xt_sb = xpool.tile([128, EC, S], BF16, tag="xt")
                nc.sync.dma_start(
                    out=xt_sb[:, :, :],
                    in_=xt[p].rearrange("(c k) s -> k c s", k=128),
                )
                # ---- Q/K projection, both packings ----
                # qk2 layout: cols [0,S) = QT, cols [S,2S) = KT, with the
                # same values on partition halves 0-63 and 64-127 so energy
                # matmuls can row-pack.  The [Q|K] and [K|Q] weight packs
                # produce the two halves directly -- no partition-crossing
                # DMA, so qk2 is written by DVE only (single-wait clean).
                qk2 = qkpool.tile([128, 2 * S], BF16, tag="qk2")
                for sq in range(NSQ):
                    ps = misc_ps.tile([128, SQT], F32, tag="misc")
                    for c in range(EC):
                        nc.tensor.matmul(
                            out=ps[:, :],
                            lhsT=wqk_sb[:, c, :],
                            rhs=xt_sb[:, c, SQT * sq : SQT * (sq + 1)],
                            start=(c == 0),
                            stop=False,
                        )
                    # bias folded in as a K=1 rank-1 update: bias x ones-row
                    nc.tensor.matmul(
                        out=ps[:, :],
                        lhsT=bqk_sb[:, :],
                        rhs=ones_row[:, :],
                        start=False,
                        stop=True,
                    )
                    ps2 = misc_ps.tile([128, SQT], F32, tag="misc")
                    for c in range(EC):
                        nc.tensor.matmul(
                            out=ps2[:, :],
                            lhsT=wkq_sb[:, c, :],
                            rhs=xt_sb[:, c, SQT * sq : SQT * (sq + 1)],
                            start=(c == 0),
                            stop=False,
                        )
                    nc.tensor.matmul(
                        out=ps2[:, :],
                        lhsT=bkq_sb[:, :],
                        rhs=ones_row[:, :],
                        start=False,
                        stop=True,
                    )
                    nc.vector.tensor_copy(
                        out=qk2[0:64, SQT * sq : SQT * (sq + 1)], in_=ps[0:64, :]
                    )
                    nc.vector.tensor_copy(
                        out=qk2[64:128, S + SQT * sq : S + SQT * (sq + 1)],
                        in_=ps[64:128, :],
                    )
                    nc.vector.tensor_copy(
                        out=qk2[0:64, S + SQT * sq : S + SQT * (sq + 1)],
                        in_=ps2[0:64, :],
                    )
                    nc.vector.tensor_copy(
                        out=qk2[64:128, SQT * sq : SQT * (sq + 1)],
                        in_=ps2[64:128, :],
                    )

                # ---- V projection -> VT [65, S] (row 64 = ones) ----
                vt_sb = vtpool.tile([65, S], BF16, tag="vt")
                nc.vector.memset(vt_sb[64:65, :], 1.0)
                for sq in range(NSQ):
                    ps = misc_ps.tile([64, SQT], F32, tag="misc")
                    for c in range(EC):
                        nc.tensor.matmul(
                            out=ps[:, :],
                            lhsT=wv_sb[:, c, :],
                            rhs=xt_sb[:, c, SQT * sq : SQT * (sq + 1)],
                            start=(c == 0),
                            stop=False,
                        )
                    nc.tensor.matmul(
                        out=ps[:, :],
                        lhsT=bv_sb[:, :],
                        rhs=ones_row[:, :],
                        start=False,
                        stop=True,
                    )
                    nc.vector.tensor_copy(
                        out=vt_sb[0:64, SQT * sq : SQT * (sq + 1)], in_=ps[:, :]
                    )

                # ---- V^T -> V [S, D+ones]: v_aug [128, 65*16]; col 64 of
                # each 65-block is the ones column (vt_sb row 64 transposed).
                # Transpose blocks land in ot-pool PSUM tiles (7+7+2) and are
                # drained to SBUF by ScalarE so attn@V's single ACT wait
                # covers them.
                v_aug = vpool.tile([128, 65 * NSK], BF16, tag="vaug")
                # 66-wide psum blocks keep bf16 PSUM writes 4-byte aligned
                vps = [
                    ot_ps.tile([128, 7, 66], BF16, tag="ot", name=f"vps0_{p}"),
                    ot_ps.tile([128, 7, 66], BF16, tag="ot", name=f"vps1_{p}"),
                    ot_ps.tile([128, 2, 66], BF16, tag="ot", name=f"vps2_{p}"),
                ]
                for t in range(NSK):
                    g, j = (0, t) if t < 7 else (1, t - 7) if t < 14 else (2, t - 14)
                    nc.tensor.transpose(
                        out=vps[g][:, j, 0:65],
                        in_=vt_sb[:, 128 * t : 128 * (t + 1)],
                        identity=identb[0:65, 0:65],
                    )
                nc.scalar.copy(
                    out=v_aug[:, 0:455].rearrange("p (t c) -> p t c", c=65),
                    in_=vps[0][:, :, 0:65],
                )
                nc.scalar.copy(
                    out=v_aug[:, 455:910].rearrange("p (t c) -> p t c", c=65),
                    in_=vps[1][:, :, 0:65],
                )
                nc.scalar.copy(
                    out=v_aug[:, 910:1040].rearrange("p (t c) -> p t c", c=65),
                    in_=vps[2][:, :, 0:65],
                )

                # ---- attention, one q-slice at a time ----
                for sq in range(NSQ):
                    otp = ot_ps.tile([65, SQT], F32, tag="ot")
                    for jj in range(NSK // 2):  # pairs of k-tiles, row-packed
                        epsum = pe_ps.tile([128, 1024], F32, tag="pe")
                        for half in range(2):
                            t = 2 * jj + half
                            base = 64 * half  # row-pack: A rows 0-63, B rows 64-127
                            nc.tensor.matmul(
                                out=epsum[:, 512 * half : 512 * (half + 1)],
                                lhsT=qk2[
                                    base : base + 64,
                                    S + 128 * t : S + 128 * (t + 1),
                                ],
                                rhs=qk2[
                                    base : base + 64,
                                    SQT * sq : SQT * (sq + 1),
                                ],
                                start=True,
                                stop=True,
                            )
                        eexp = epool.tile([128, 1024], BF16, tag="expe")
                        nc.scalar.activation(
                            out=eexp[:, :],
                            in_=epsum[:, :],
                            func=mybir.ActivationFunctionType.Exp,
                        )
                        for half in range(2):
                            t = 2 * jj + half
                            nc.tensor.matmul(
                                out=otp[:, :],
                                lhsT=v_aug[:, 65 * t : 65 * (t + 1)],
                                rhs=eexp[:, 512 * half : 512 * (half + 1)],
                                start=(t == 0),
                                stop=(t == NSK - 1),
                            )

                    # ---- transpose + normalize + store ----
                    ot_sb = opool.tile([65, SQT], F32, tag="osb")
                    nc.vector.tensor_copy(out=ot_sb[:, :], in_=otp[:, :])
                    fin = fpool.tile([128, NSQ * D], F32, tag="fin")
                    nc.vector.memset(fin[:, :], 0.0)
                    for b in range(SQT // 128):
                        pt = misc_ps.tile([128, 65], F32, tag="misc")
                        nc.tensor.transpose(
                            out=pt[:, :],
                            in_=ot_sb[:, 128 * b : 128 * (b + 1)],
                            identity=ident[0:65, 0:65],
                        )
                        rec = spool.tile([128, 1], F32, tag="stat")
                        nc.vector.reciprocal(out=rec[:, :], in_=pt[:, 64:65])
                        nc.scalar.activation(
                            out=fin[:, D * b : D * (b + 1)],
                            in_=pt[:, 0:64],
                            func=mybir.ActivationFunctionType.Copy,
                            scale=rec[:, :],
                        )
                    nc.sync.dma_start(
                        out=out[p, SQT * sq : SQT * (sq + 1), :].rearrange(
                            "(b r) d -> r b d", r=128
                        ),
                        in_=fin[:, :].rearrange("r (b d) -> r b d", d=D),
                    )
    return nc


def _prep_inputs(x, Wq, bq, Wk, bk, Wv, bv):
    import ml_dtypes

    bf16 = ml_dtypes.bfloat16
    scale = 1.0 / np.sqrt(np.float32(E))
    # fold softmax scale into Wq/bq; pack Q|K weights for the fused proj
    wqk = np.concatenate([Wq * scale, Wk], axis=1).astype(bf16)
    bqk = np.concatenate([bq * scale, bk]).astype(bf16).reshape(1, 128)
    wkq = np.concatenate([Wk, Wq * scale], axis=1).astype(bf16)
    bkq = np.concatenate([bk, bq * scale]).astype(bf16).reshape(1, 128)
    wv = np.ascontiguousarray(Wv.astype(bf16))
    bvc = bv.astype(bf16).reshape(1, D)
    # x [N,S,H,E] -> per-(n,h) transposed [E,S]; pair index p = n*H + h
    xt_all = np.ascontiguousarray(
        x.astype(bf16).transpose(0, 2, 3, 1)
    ).reshape(N * H, E, S)
    in_maps = []
    for core in range(NCORES):
        in_maps.append(
            {
                "xt": np.ascontiguousarray(xt_all[PAIRS * core : PAIRS * (core + 1)]),
                "wqk": wqk,
                "bqk": bqk,
                "wkq": wkq,
                "bkq": bkq,
                "wv": wv,
                "bv": bvc,
            }
        )
    return in_maps


def _gather(results):
    out = np.empty((N, S, H, D), dtype=np.float32)
    for core in range(NCORES):
        for j in range(PAIRS):
            p = PAIRS * core + j
            out[p // H, :, p % H, :] = results[core]["out"][j]
    return out


def _finalize(nc):
    import bass_rust

    nc.finalize()
    # Bacc's in-finalize pass ordering leaves Tile-emitted multi-wait sync
    # sets intact; walrus codegen rejects >1 sync wait on matmult (and some
    # DVE structs).  A second generate_event_semaphores pass splits them.
    bass_rust.generate_event_semaphores(nc)
    return nc


def kernel(x, Wq, bq, Wk, bk, Wv, bv):
    nc = _finalize(build_bass())
    in_maps = _prep_inputs(x, Wq, bq, Wk, bk, Wv, bv)
    res = run_bass_kernel_spmd(nc, in_maps, list(range(NCORES)))
    return _gather(res.results)
